# revision 20
# baseline (speedup 1.0000x reference)
"""DigitCapsule dynamic-routing kernel for 8 Trainium2 NeuronCores.

Key restructuring: u_hat (B,R,D,O) = 188 MB is NEVER materialized.
  s[b,(d,o)]  = sum_{(r,i)} (c[r,d]*W[r,d,o,i]) * u[b,r,i]      (matmul over (r,i))
  b_upd[r,d]  = sum_{i,o} W[r,d,o,i] * G[(r,i),(d,o)],
  G[(r,i),(d,o)] = sum_b u[b,(r,i)] * v[b,(d,o)]                 (matmul over b)

Sharding: route nodes R=1152 are split 144/core across 8 cores.  Softmax
(over d) and the b-logit update are then fully local; the only collective
is one AllReduce of the partial s per routing iteration (3 total).

Perf notes (v5):
  * All PE operands are bf16 (fp32 matmuls cost 4 cycles/row vs bf16's 1);
    the AllReduce payload is bf16 too, so its output feeds mm2 directly.
    PSUM accumulation and the squash/logit math stay fp32.
  * The routing logits LIVE IN PSUM: the per-group J matmul accumulates
    g_k * (J @ Hred_k) across iterations (J is pre-scaled by the squash
    scalar each iteration), so there are no logit read-modify-writes on
    the DVE and the softmax Exp reads the PSUM bank directly.
  * mm2's G drains PSUM->bf16 on the Act engine so the W*G multiply runs
    in the DVE 2x 16-bit mode; mm1's s drains ride the Act engine too.
  * The collective input is written by two half DMAs (SP for the early
    half, Pool for the late one); the collective chain stays on the Pool
    sequencer.  u_nat+J loads are gated past the uT/Wp loads so nothing
    contends with the AR window on the serialized DMA transfer engine.
  * Softmax is batched (one Exp / reduce / reciprocal / multiply); CW is
    built in 3 chunks (DVE, DVE, Pool) and mm1 accumulates t-tiles in
    chunk-arrival order (6..8, 0..2, 3..5).
The device tracks s_dev = A*s_true (A=1 normally; iteration 0 skips the
softmax entirely, feeding W straight to mm1, so A = 10 there) and corrects
inside squash: v = s_dev * sqrt(T)/(A^2 + T) with T = sum(s_dev^2).
"""

import ml_dtypes
import numpy as np

import concourse.bass as bass
import concourse.mybir as mybir
import concourse.tile as tile
from concourse.bass_utils import run_bass_kernel_spmd
from concourse.tile import add_dep_helper

N_CORES = 8
B, R, D, O, I_CH = 256, 1152, 10, 16, 8
RL = R // N_CORES           # 144 route nodes per core
KRI = RL * I_CH             # 1152 = (r,i) contraction length per core
NT = KRI // 128             # 9 partition tiles of (r,i)
DO = D * O                  # 160
NB = B // 128               # 2 batch halves
N_ITER = 3

f32 = mybir.dt.float32
bf16 = mybir.dt.bfloat16
ALU = mybir.AluOpType
AF = mybir.ActivationFunctionType

_ws_ctr = [0]


def _split_excess_waits(nc, max_waits=1):
    """Walrus in this container only lowers one sync-wait per instruction.
    Hoist excess waits onto NOPs inserted before the instruction on the
    same engine (same-order execution => identical semantics)."""
    n_split = 0
    for f in nc.m.functions:
        for bb in f.blocks:
            out = []
            changed = False
            for ins in bb.instructions:
                si = ins.sync_info
                waits = list(si.on_wait) if (si is not None and si.on_wait) else []
                if len(waits) > max_waits:
                    changed = True
                    n_split += 1
                    head, rest = waits[:-max_waits], waits[-max_waits:]
                    while head:
                        chunk, head = head[:max_waits], head[max_waits:]
                        _ws_ctr[0] += 1
                        nop = mybir.InstNoOp(name=f"I-ws{_ws_ctr[0]}")
                        nop.engine = ins.engine
                        nop.sync_info = mybir.SyncInfo(on_wait=chunk, on_update=[])
                        out.append(nop)
                    ins.sync_info = mybir.SyncInfo(
                        on_wait=rest,
                        on_update=list(si.on_update) if si.on_update else [],
                    )
                out.append(ins)
            if changed:
                bb.instructions = out
    return n_split


def _build_nc(reps=1, prewarm=10):
    nc = bass.Bass(
        "TRN2", target_bir_lowering=False, debug=False, num_devices=N_CORES
    )
    un_d = nc.dram_tensor("u_nat", [NB, 128, KRI], bf16, kind="ExternalInput")
    uT_d = nc.dram_tensor("uT", [128, NT, B], bf16, kind="ExternalInput")
    Wp_d = nc.dram_tensor("Wp", [128, NT, DO], bf16, kind="ExternalInput")
    Jm_d = nc.dram_tensor("Jm", [128, 128], f32, kind="ExternalInput")
    v_out_d = nc.dram_tensor("v_out", [NB, 128, DO], f32, kind="ExternalOutput")

    rg = [list(range(N_CORES))]

    with tile.TileContext(nc) as tc:
        with (
            tc.tile_pool(name="persist", bufs=1) as pp_,
            tc.tile_pool(name="iter", bufs=2) as ip_,
            tc.tile_pool(name="small", bufs=2) as sp_,
            tc.tile_pool(name="dram", bufs=2, space="DRAM") as dp_,
            tc.tile_pool(name="ps_s", bufs=1, space="PSUM") as ps_s,
            tc.tile_pool(name="ps_g", bufs=2, space="PSUM") as ps_g,
            tc.tile_pool(name="ps_b", bufs=1, space="PSUM") as ps_b,
            tc.tile_pool(name="ps_t", bufs=1, space="PSUM") as ps_t,
        ):
            # ---- persistent tensors ----
            un = pp_.tile([128, NB, KRI], bf16)
            uT = pp_.tile([128, NT, B], bf16)
            Wp = pp_.tile([128, NT, DO], bf16)
            J = pp_.tile([128, 128], f32)
            ones = pp_.tile([128, 128], f32)
            ones16 = pp_.tile([128, 128], bf16)
            # routing logits accumulate in PSUM across iterations
            bd_acc = pp_.tile([128, NT * D], f32, name="bd_acc")

            # uT+Wp gate mm1 of iteration 0 -> loaded first on the SP and
            # Act queues so mm1 starts on the first chunks.  Chunks keep
            # the full 256-wide b axis so the innermost contiguous run is
            # >= 512B (avoids the 2x DMA penalty).
            last_ld = None
            for lo, hi in ((0, 5), (5, 9)):
                last_ld = nc.sync.dma_start(uT[:, lo:hi, :], uT_d[:, lo:hi, :])
                nc.scalar.dma_start(Wp[:, lo:hi, :], Wp_d[:, lo:hi, :])
            nc.gpsimd.memset(ones[:], 1.0)
            nc.gpsimd.memset(ones16[:], 1.0)
            # Warm the PE clock while the uT/Wp DMAs are in flight.
            if prewarm:
                pw_ps = ps_t.tile([128, 128], f32, name="pw", tag="wm")
                for k in range(prewarm):
                    nc.tensor.matmul(
                        pw_ps[:], ones16[:], ones16[:], start=True, stop=True
                    )
            # u_nat / J are not needed until mm2 (~20us in); gate them on
            # the last uT chunk so their transfers run after the uT/Wp
            # window but before the AR0 input hits the serialized DMA
            # transfer engine.  They ride the SP queue (free after the uT
            # chunks) so the Act queue can pick up the s drains promptly.
            half = KRI // 2
            for q0 in range(0, KRI, half):
                d = nc.sync.dma_start(
                    un[:, 0, q0 : q0 + half], un_d[0, :, q0 : q0 + half]
                )
                add_dep_helper(d.ins, last_ld.ins, sync=True,
                               reason="defer u_nat past uT/Wp")

            def _emit_late_loads(anchor):
                for q0 in range(0, KRI, half):
                    d = nc.sync.dma_start(
                        un[:, 1, q0 : q0 + half], un_d[1, :, q0 : q0 + half]
                    )
                    add_dep_helper(d.ins, anchor.ins, sync=True,
                                   reason="defer u_nat past AR0 input")
                dj = nc.sync.dma_start(J[:], Jm_d[:])
                add_dep_helper(dj.ins, anchor.ins, sync=True,
                               reason="defer J past AR0 input")

            for it in range(N_ITER * reps):
                rep, it = divmod(it, N_ITER)
                last = it == N_ITER - 1
                if it == 0:
                    # b==0 => c uniform: feed W directly, fold 1/(10*16)
                    # into the squash constants (s_dev = 10 * s_true).
                    CW = Wp
                    A2 = 100.0
                else:
                    # CW was produced at the end of the previous iteration
                    CW = CW_next
                    A2 = 1.0
                mm1_order = list(range(NT))
                # ---- mm1: s_dev[b,(d,o)] = sum_(r,i) uT.T @ CW ----
                s_sb = ip_.tile([128, NB, DO], bf16, name=f"s{rep}_{it}", tag="s")
                inb = dp_.tile([128, NB * DO], bf16, name=f"inb{rep}_{it}", tag="inb")
                outb = dp_.tile(
                    [128, NB * DO], bf16, name=f"outb{rep}_{it}", tag="outb",
                    addr_space="Shared",
                )
                s_ps = [
                    ps_s.tile(
                        [128, DO], f32, name=f"sps{rep}_{it}_{h}", tag=f"sps{h}"
                    )
                    for h in range(NB)
                ]
                for k, t in enumerate(mm1_order):
                    for h in range(NB):
                        nc.tensor.matmul(
                            s_ps[h][:],
                            uT[:, t, h * 128 : (h + 1) * 128],
                            CW[:, t, :],
                            start=(k == 0),
                            stop=(k == NT - 1),
                        )
                for h in range(NB):
                    # drain on the Act engine (bf16 convert); the half-DMA
                    # into the collective input overlaps the other half.
                    nc.scalar.activation(s_sb[:, h, :], s_ps[h][:], AF.Copy)
                    eng = nc.sync if h == 0 else nc.gpsimd
                    dma = eng.dma_start(inb[:, h * DO : (h + 1) * DO], s_sb[:, h, :])
                    if h == 0 and it == 0 and rep == 0:
                        _emit_late_loads(dma)
                # ---- AllReduce partial s (bf16) over the 8 cores ----
                nc.gpsimd.collective_compute(
                    "AllReduce", ALU.add, replica_groups=rg,
                    ins=[inb.opt()], outs=[outb.opt()],
                )
                sf = ip_.tile([128, NB, DO], bf16, name=f"sf{rep}_{it}", tag="sf")
                nc.sync.dma_start(sf[:].rearrange("p h f -> p (h f)"), outb[:])

                # ---- squash with global norm over the full batch ----
                # s_dev = A*s_true  =>  v = s_dev * sqrt(T)/(A^2 + T),
                # T = sum(s_dev^2).  Split in two parts so the DVE ops can
                # be emitted after the first mm2 group (no head blocking).
                def emit_squash_front(rep=rep, it=it, sf=sf):
                    sqscr = sp_.tile(
                        [128, NB * DO], f32, name=f"sq{rep}_{it}", tag="sq"
                    )
                    ppsum = sp_.tile([128, 1], f32, name=f"pps{rep}_{it}", tag="pps")
                    nc.scalar.activation(
                        sqscr[:], sf[:].rearrange("p h f -> p (h f)"), AF.Square,
                        accum_out=ppsum[:],
                    )
                    T_ps = ps_t.tile([128, 1], f32, name=f"T{rep}_{it}", tag="wm")
                    nc.tensor.matmul(
                        T_ps[:], ones[:], ppsum[:], start=True, stop=True
                    )
                    return T_ps

                def emit_squash_back(T_ps, rep=rep, it=it, A2=A2):
                    q = sp_.tile([128, 1], f32, name=f"q{rep}_{it}", tag="q")
                    nc.vector.tensor_scalar_add(q[:], T_ps[:], A2)
                    qinv = sp_.tile([128, 1], f32, name=f"qi{rep}_{it}", tag="qi")
                    nc.vector.reciprocal(qinv[:], q[:])
                    rt = sp_.tile([128, 1], f32, name=f"rt{rep}_{it}", tag="rt")
                    nc.scalar.activation(rt[:], T_ps[:], AF.Sqrt)
                    g = sp_.tile([128, 1], f32, name=f"g{rep}_{it}", tag="g")
                    nc.vector.tensor_tensor(g[:], rt[:], qinv[:], op=ALU.mult)
                    return g

                if last:
                    g = emit_squash_back(emit_squash_front())
                    v_sb = ip_.tile([128, NB, DO], f32, name=f"v{rep}_{it}", tag="v")
                    nc.vector.tensor_scalar_mul(
                        v_sb[:].rearrange("p h f -> p (h f)"),
                        sf[:].rearrange("p h f -> p (h f)"),
                        g[:, 0:1],
                    )
                    nc.sync.dma_start(
                        v_out_d[:].rearrange("h p f -> p h f"), v_sb[:]
                    )
                else:
                    # ---- mm2 on sf directly (G = u.T@sf); the squash
                    # scalar g folds into the J matmul, and the logit
                    # update accumulates in the bd_acc PSUM bank ----
                    Hred = ip_.tile([128, NT, D], f32, name=f"hr{rep}_{it}", tag="hr")
                    groups = [(0, 2), (2, 4), (4, 6), (6, 8), (8, 9)]
                    e = ip_.tile([128, 8, D], f32, name=f"e{rep}_{it}", tag="e")
                    den = ip_.tile([128, 8], f32, name=f"den{rep}_{it}", tag="den")
                    rec = ip_.tile([128, 8], f32, name=f"rc{rep}_{it}", tag="rc")
                    cc = ip_.tile([128, 8, D], f32, name=f"c{rep}_{it}", tag="c")
                    e8 = ip_.tile([128, D], f32, name=f"e8{rep}_{it}", tag="e8")
                    den8 = ip_.tile([128, 1], f32, name=f"dn8{rep}_{it}", tag="dn8")
                    rec8 = ip_.tile([128, 1], f32, name=f"rc8{rep}_{it}", tag="rc8")
                    CW_next = ip_.tile(
                        [128, NT, DO], bf16, name=f"cw{rep}_{it}", tag="cw"
                    )

                    def emit_G(lo, hi, rep=rep, it=it, sf=sf):
                        G_ps = ps_g.tile(
                            [128, hi - lo, DO], f32,
                            name=f"G{rep}_{it}_{lo}", tag="G",
                        )
                        for k, t in enumerate(range(lo, hi)):
                            for h in range(NB):
                                nc.tensor.matmul(
                                    G_ps[:, k, :],
                                    un[:, h, t * 128 : (t + 1) * 128],
                                    sf[:, h, :],
                                    start=(h == 0),
                                    stop=(h == NB - 1),
                                )
                        return G_ps

                    G_pre = emit_G(*groups[0])
                    T_ps = emit_squash_front()
                    g = None
                    Jg = ip_.tile([128, 128], f32, name=f"Jg{rep}_{it}", tag="Jg")
                    for gi, (lo, hi) in enumerate(groups):
                        n = hi - lo
                        G_ps = G_pre if gi == 0 else emit_G(lo, hi)
                        if gi < len(groups) - 1:
                            # drain G to bf16 on the Act engine so the W*G
                            # multiply runs in DVE 2x 16-bit mode
                            G16 = sp_.tile(
                                [128, n, DO], bf16,
                                name=f"g16{rep}_{it}_{lo}", tag="g16",
                            )
                            nc.scalar.activation(G16[:], G_ps[:], AF.Copy)
                            Ht = sp_.tile(
                                [128, n, DO], bf16,
                                name=f"ht{rep}_{it}_{lo}", tag="ht",
                            )
                            nc.vector.tensor_tensor(
                                Ht[:], G16[:], Wp[:, lo:hi, :], op=ALU.mult
                            )
                        else:
                            # last (small) group: skip the Act hop
                            Ht = sp_.tile(
                                [128, n, DO], f32,
                                name=f"ht{rep}_{it}_{lo}", tag="ht",
                            )
                            nc.vector.tensor_tensor(
                                Ht[:], G_ps[:], Wp[:, lo:hi, :], op=ALU.mult
                            )
                        nc.vector.reduce_sum(
                            Hred[:, lo:hi, :],
                            Ht[:].rearrange("p t (d o) -> p t d o", d=D, o=O),
                            axis=mybir.AxisListType.X,
                        )
                        if gi == 0:
                            # g-chain DVE ops emitted here: they never
                            # block the mm2 pipeline's DVE queue head
                            g = emit_squash_back(T_ps)
                            # fold g into the i-sum matmul's stationary
                            nc.gpsimd.tensor_scalar_mul(Jg[:], J[:], g[:, 0:1])
                        # i-sum + broadcast + logit accumulation in PSUM
                        bd_ps = ps_g.tile(
                            [128, n * D], f32, name=f"bdp{rep}_{it}_{lo}", tag="bdx"
                        )
                        nc.tensor.matmul(
                            bd_ps[:], Jg[:], Hred[:, lo:hi, :],
                            start=True, stop=True,
                        )
                        if it == 0:
                            nc.scalar.activation(
                                bd_acc[:, lo * D : hi * D], bd_ps[:], AF.Copy
                            )
                        else:
                            nc.vector.tensor_tensor(
                                bd_acc[:, lo * D : hi * D],
                                bd_acc[:, lo * D : hi * D], bd_ps[:],
                                op=ALU.add,
                            )
                        if gi == len(groups) - 2:
                            # t0..7 logits are final: batched softmax and
                            # the bulk CW chunks for the NEXT iteration,
                            # overlapping the t8 group's agreement path.
                            nc.scalar.activation(
                                e[:],
                                bd_acc[:, 0 : 8 * D].rearrange(
                                    "p (t d) -> p t d", t=8, d=D
                                ),
                                AF.Exp,
                            )
                            nc.vector.reduce_sum(
                                den[:].unsqueeze(2), e[:],
                                axis=mybir.AxisListType.X,
                            )
                            nc.vector.reciprocal(
                                rec[:].unsqueeze(2), den[:].unsqueeze(2)
                            )
                            nc.vector.tensor_tensor(
                                cc[:], e[:],
                                rec[:].unsqueeze(2).broadcast_to([128, 8, D]),
                                op=ALU.mult,
                            )
                            for ci, (lo2, hi2) in enumerate(((0, 4), (4, 8))):
                                eng = nc.vector if ci == 0 else nc.gpsimd
                                eng.tensor_tensor(
                                    CW_next[:, lo2:hi2, :].rearrange(
                                        "p t (d o) -> p t d o", d=D, o=O
                                    ),
                                    Wp[:, lo2:hi2, :].rearrange(
                                        "p t (d o) -> p t d o", d=D, o=O
                                    ),
                                    cc[:, lo2:hi2, :].unsqueeze(3).broadcast_to(
                                        [128, hi2 - lo2, D, O]
                                    ),
                                    op=ALU.mult,
                                )
                        if gi == len(groups) - 1:
                            # t8 scalar tail: exp with denominator
                            # accumulator, reciprocal, one fused stt
                            nc.scalar.activation(
                                e8[:], bd_acc[:, 8 * D : 9 * D], AF.Exp,
                                accum_out=den8[:],
                            )
                            nc.vector.reciprocal(rec8[:], den8[:])
                            nc.vector.scalar_tensor_tensor(
                                CW_next[:, 8, :].rearrange(
                                    "p (d o) -> p d o", d=D, o=O
                                ),
                                Wp[:, 8, :].rearrange(
                                    "p (d o) -> p d o", d=D, o=O
                                ),
                                rec8[:, 0:1],
                                e8[:].unsqueeze(2).broadcast_to([128, D, O]),
                                op0=ALU.mult, op1=ALU.mult,
                            )

    _split_excess_waits(nc, 1)
    return nc


_NC_CACHE = {}


def _get_nc(reps=1):
    key = (reps,)
    if key not in _NC_CACHE:
        _NC_CACHE[key] = _build_nc(reps=reps)
    return _NC_CACHE[key]


def _prep_core_inputs(u, W, c):
    r0, r1 = c * RL, (c + 1) * RL
    u2 = np.ascontiguousarray(u[:, r0:r1, :]).reshape(B, KRI)
    u_nat = np.ascontiguousarray(u2.reshape(NB, 128, KRI)).astype(ml_dtypes.bfloat16)
    uT = np.ascontiguousarray(
        np.ascontiguousarray(u2.T).reshape(NT, 128, B).transpose(1, 0, 2)
    ).astype(ml_dtypes.bfloat16)
    Wp2 = np.ascontiguousarray(W[0, r0:r1].transpose(0, 3, 1, 2)).reshape(KRI, DO)
    Wp = np.ascontiguousarray(
        Wp2.reshape(NT, 128, DO).transpose(1, 0, 2)
    ).astype(ml_dtypes.bfloat16)
    return {"u_nat": u_nat, "uT": uT, "Wp": Wp}


def kernel(u, W, _trace=False, _reps=1):
    u = np.asarray(u, dtype=np.float32)
    W = np.asarray(W, dtype=np.float32)
    assert u.shape == (B, R, I_CH) and W.shape == (1, R, D, O, I_CH)
    Jm = np.kron(np.eye(16, dtype=np.float32), np.ones((8, 8), np.float32))
    in_maps = []
    for c in range(N_CORES):
        m = _prep_core_inputs(u, W, c)
        m["Jm"] = Jm
        in_maps.append(m)
    nc = _get_nc(_reps)
    res = run_bass_kernel_spmd(
        nc, in_maps, core_ids=list(range(N_CORES)), trace=_trace
    )
    v = res.results[0]["v_out"].reshape(B, D, O).astype(np.float32)
    if _trace:
        return v, res
    return v
